# revision 14
# baseline (speedup 1.0000x reference)
"""Trainium2 Bass kernel for nn_MultiHeadAttention (B=4, S=2048, D=1024, H=16).

Sharding: 8 cores = (batch b, qrow-half). Each core computes a full MHA block
for its 1024 query rows of batch b: QKV projections, dense softmax attention
(returning the full attention-weight matrix), output projection, residual +
LayerNorm. No collectives; host concatenates shards.
"""
import contextlib
import ctypes
import os
import sys
import types

sys.path.insert(0, "/opt/trn_rl_repo")

import numpy as np

B, S, D, H, DK = 4, 2048, 1024, 16, 64
ROWS = 1024          # q rows per core
NCORES = 8
LN_EPS = 1e-5

_cache = {}


# ---------------------------------------------------------------------------
# NTFF profile hook (so BASS_TRACE=1 works under axon); degrades silently.
# ---------------------------------------------------------------------------
def _install_profile_hook():
    try:
        import antenv
        if "antenv.axon_hooks" in sys.modules:
            return
        store = {"hook": None}

        def _make_hook(so_path):
            try:
                lib = ctypes.CDLL(so_path)
            except OSError:
                return None
            if not hasattr(lib, "axon_start_nrt_profile"):
                return None
            lib.axon_start_nrt_profile.argtypes = [
                ctypes.POINTER(ctypes.c_int64), ctypes.c_size_t]
            lib.axon_start_nrt_profile.restype = ctypes.c_int64
            lib.axon_stop_nrt_profile.argtypes = [ctypes.c_char_p]
            lib.axon_stop_nrt_profile.restype = ctypes.c_int64

            @contextlib.contextmanager
            def _hook(output_dir, device_ids):
                import jax
                jax.devices()
                if device_ids:
                    ids = (ctypes.c_int64 * len(device_ids))(*device_ids)
                    rc = lib.axon_start_nrt_profile(ids, len(device_ids))
                else:
                    rc = lib.axon_start_nrt_profile(None, 0)
                if rc != 0:
                    raise RuntimeError(f"axon_start_nrt_profile rc={rc}")
                try:
                    yield
                finally:
                    lib.axon_stop_nrt_profile(str(output_dir).encode())

            return _hook

        mod = types.ModuleType("antenv.axon_hooks")
        mod.set_axon_ntff_profile_hook = lambda h: store.__setitem__("hook", h)
        mod.get_axon_ntff_profile_hook = lambda: store["hook"]
        sys.modules["antenv.axon_hooks"] = mod
        antenv.axon_hooks = mod
        store["hook"] = _make_hook("/opt/axon/libaxon_pjrt.so")
    except Exception:
        pass


def _build():
    import concourse.bass as bass
    import concourse.mybir as mybir
    import concourse.tile as tile
    from concourse import bacc
    from contextlib import ExitStack

    f32 = mybir.dt.float32
    bf16 = mybir.dt.bfloat16
    Act = mybir.ActivationFunctionType

    nc = bacc.Bacc("TRN2", target_bir_lowering=False, debug=False)

    q_d = nc.dram_tensor("q", [ROWS, D], f32, kind="ExternalInput").ap()
    k_d = nc.dram_tensor("k", [S, D], f32, kind="ExternalInput").ap()
    v_d = nc.dram_tensor("v", [S, D], f32, kind="ExternalInput").ap()
    W_d = {w: nc.dram_tensor(w, [D, D], f32, kind="ExternalInput").ap()
           for w in ("Wq", "Wk", "Wv", "Wo")}
    b_d = {w: nc.dram_tensor(w, [1, D], f32, kind="ExternalInput").ap()
           for w in ("bq", "bk", "bv", "bo")}
    gamma_d = nc.dram_tensor("gamma", [1, D], f32, kind="ExternalInput").ap()
    beta_d = nc.dram_tensor("beta", [1, D], f32, kind="ExternalInput").ap()
    out_d = nc.dram_tensor("out", [ROWS, D], f32, kind="ExternalOutput").ap()
    aw_d = nc.dram_tensor("attn_w", [H, ROWS, S], f32, kind="ExternalOutput").ap()

    # bf16 bounce copies of the inputs (for xbar DMA transpose, 2-byte only)
    qb_d = nc.dram_tensor("qb16", [ROWS, D], bf16).ap()
    rs_d = nc.dram_tensor("rs_scratch", [H, ROWS], f32).ap()
    rc_d = nc.dram_tensor("rc_scratch", [H, ROWS], bf16).ap()
    kb_d = nc.dram_tensor("kb16", [S, D], bf16).ap()
    vb_d = nc.dram_tensor("vb16", [S, D], bf16).ap()

    with tile.TileContext(nc) as tc, ExitStack() as ctx:
        const = ctx.enter_context(tc.tile_pool(name="const", bufs=1))
        resid = ctx.enter_context(tc.tile_pool(name="resid", bufs=1))

        ones = const.tile([1, 512], bf16)
        nc.vector.memset(ones[:], 1.0)
        ones_f32 = const.tile([1, 1], f32)
        nc.vector.memset(ones_f32[:], 1.0)
        gamma_bc = const.tile([128, D], f32)
        nc.gpsimd.dma_start(gamma_bc[:], gamma_d.to_broadcast([128, D]))
        beta_bc = const.tile([128, D], f32)
        nc.gpsimd.dma_start(beta_bc[:], beta_d.to_broadcast([128, D]))
        bias_b = {}
        for w in ("bq", "bk", "bv", "bo"):
            t = const.tile([1, D], bf16, tag=w, name=f"{w}_b")
            nc.gpsimd.dma_start(t[:], b_d[w][:])   # cast f32->bf16
            bias_b[w] = t

        # cast inputs to bf16 in HBM (SWDGE cast, HBM->HBM)
        nc.gpsimd.dma_start(qb_d[:], q_d[:])
        nc.gpsimd.dma_start(kb_d[:], k_d[:])
        nc.gpsimd.dma_start(vb_d[:], v_d[:])

        # resident projected tensors
        qhT = [resid.tile([128, ROWS], bf16, tag=f"qhT{i}", name=f"qhT{i}") for i in range(8)]
        khT = [resid.tile([128, S], bf16, tag=f"khT{i}", name=f"khT{i}") for i in range(8)]
        vh = [resid.tile([128, H * 65], bf16, tag=f"vh{i}", name=f"vh{i}") for i in range(16)]
        attnT = [resid.tile([64, ROWS], bf16, tag=f"attnT{i}", name=f"attnT{i}") for i in range(16)]

        # ============== Phase A: projections ==============
        with tc.tile_pool(name="pA", bufs=10) as pA, \
             tc.tile_pool(name="pAw", bufs=9) as pAw, \
             tc.tile_pool(name="psA", bufs=3, space="PSUM") as psA:

            ctx_q = nc.named_scope("A_qproj"); ctx_q.__enter__()
            # --- Q: qhT[m] = (Wq[:, m].T @ q.T + bq) : [128 dims, ROWS] ---
            Wq_t = []
            for dc in range(8):
                t = pAw.tile([128, D], bf16, tag="wx", name=f"wq{dc}")
                nc.gpsimd.dma_start(t[:], W_d["Wq"][128 * dc:128 * dc + 128, :])
                Wq_t.append(t)
            for rs in range(ROWS // 512):
                qTs = []
                for dc in range(8):
                    t = pA.tile([128, 512], bf16, tag="xT", name=f"qT_{rs}_{dc}")
                    nc.sync.dma_start(
                        t[:], qb_d[512 * rs:512 * rs + 512, 128 * dc:128 * dc + 128],
                        transpose=True)
                    qTs.append(t)
                for m in range(8):
                    ps = psA.tile([128, 1024], f32, tag="psa")
                    for dc in range(8):
                        nc.tensor.matmul(
                            ps[:, 0:512], Wq_t[dc][:, 128 * m:128 * m + 128],
                            qTs[dc][:, 0:512], start=(dc == 0), stop=False)
                    nc.tensor.matmul(
                        ps[:, 0:512], bias_b["bq"][:, 128 * m:128 * m + 128],
                        ones[:, 0:512], start=False, stop=True)
                    nc.vector.tensor_copy(
                        qhT[m][:, 512 * rs:512 * rs + 512], ps[:, 0:512])

            ctx_q.__exit__(None, None, None)
            ctx_k = nc.named_scope("A_kproj"); ctx_k.__enter__()
            # --- K: khT[m] = (Wk[:, m].T @ k.T + bk) : [128 dims, S] ---
            Wk_t = []
            for dc in range(8):
                t = pAw.tile([128, D], bf16, tag="wx", name=f"wk{dc}")
                nc.gpsimd.dma_start(t[:], W_d["Wk"][128 * dc:128 * dc + 128, :])
                Wk_t.append(t)
            for ks in range(S // 512):
                kTs = []
                for dc in range(8):
                    t = pA.tile([128, 512], bf16, tag="xT", name=f"kT_{ks}_{dc}")
                    nc.sync.dma_start(
                        t[:], kb_d[512 * ks:512 * ks + 512, 128 * dc:128 * dc + 128],
                        transpose=True)
                    kTs.append(t)
                for m in range(8):
                    ps = psA.tile([128, 1024], f32, tag="psa")
                    for dc in range(8):
                        nc.tensor.matmul(
                            ps[:, 0:512], Wk_t[dc][:, 128 * m:128 * m + 128],
                            kTs[dc][:, 0:512], start=(dc == 0), stop=False)
                    nc.tensor.matmul(
                        ps[:, 0:512], bias_b["bk"][:, 128 * m:128 * m + 128],
                        ones[:, 0:512], start=False, stop=True)
                    nc.vector.tensor_copy(
                        khT[m][:, 512 * ks:512 * ks + 512], ps[:, 0:512])

            ctx_k.__exit__(None, None, None)
            ctx_v = nc.named_scope("A_vproj"); ctx_v.__enter__()
            # --- V: vh[kb][key, 65h+j] = (k? no: v @ Wv + bv), ones col at 65h+64
            Wv_t = []
            for dc in range(8):
                t = pAw.tile([128, D], bf16, tag="wx", name=f"wv{dc}")
                nc.gpsimd.dma_start(t[:], W_d["Wv"][128 * dc:128 * dc + 128, :])
                Wv_t.append(t)
            for ks in range(S // 512):
                vTs = []
                for dc in range(8):
                    t = pA.tile([128, 512], bf16, tag="xT", name=f"vT_{ks}_{dc}")
                    nc.sync.dma_start(
                        t[:], vb_d[512 * ks:512 * ks + 512, 128 * dc:128 * dc + 128],
                        transpose=True)
                    vTs.append(t)
                for j in range(4):           # key blocks within this slice
                    kb = 4 * ks + j
                    ps = psA.tile([128, 1024], f32, tag="psa")
                    for half in range(2):
                        sl = slice(512 * half, 512 * half + 512)
                        for dc in range(8):
                            nc.tensor.matmul(
                                ps[:, sl], vTs[dc][:, 128 * j:128 * j + 128],
                                Wv_t[dc][:, sl], start=(dc == 0), stop=False)
                        nc.tensor.matmul(
                            ps[:, sl], ones[:, 0:128], bias_b["bv"][:, sl],
                            start=False, stop=True)
                    # scatter into vh[kb]: per head 64 cols + ones col
                    for h in range(H):
                        nc.vector.tensor_copy(
                            vh[kb][:, 65 * h:65 * h + 64],
                            ps[:, 64 * h:64 * h + 64])
                    ones_col = vh[kb][:].rearrange(
                        "p (h c) -> p h c", c=65)[:, :, 64:65]
                    nc.vector.memset(ones_col, 1.0)

            ctx_v.__exit__(None, None, None)

        # ============== Phase B: attention ==============
        with tc.tile_pool(name="pB", bufs=4) as pB, \
             tc.tile_pool(name="pBr", bufs=2) as pBr, \
             tc.tile_pool(name="psT", bufs=1, space="PSUM") as psTp, \
             tc.tile_pool(name="psN", bufs=2, space="PSUM") as psNp, \
             tc.tile_pool(name="psAt", bufs=1, space="PSUM") as psAt:

            for h in range(H):
                p, st = h // 2, 64 * (h % 2)
                # ---- T side: scoresT -> expT -> PV (+rowsum via ones col) ----
                at = psAt.tile([65, ROWS], f32, tag="attn")
                ctx_t = nc.named_scope("B_Tside"); ctx_t.__enter__()
                for kb in range(16):
                    ps = psTp.tile([128, ROWS], f32, tag="scT")
                    for qs in range(2):
                        nc.tensor.matmul(
                            ps[:, 512 * qs:512 * qs + 512],
                            khT[p][st:st + 64, 128 * kb:128 * kb + 128],
                            qhT[p][st:st + 64, 512 * qs:512 * qs + 512],
                            start=True, stop=True)
                    et = pB.tile([128, ROWS], bf16, tag="expT")
                    nc.scalar.activation(et[:], ps[:], Act.Exp, scale=0.125)
                    for qs in range(2):
                        nc.tensor.matmul(
                            at[0:65, 512 * qs:512 * qs + 512],
                            vh[kb][:, 65 * h:65 * h + 65],
                            et[:, 512 * qs:512 * qs + 512],
                            start=(kb == 0), stop=(kb == 15))
                ctx_t.__exit__(None, None, None)
                # evac: attn rows -> attnT[h]; rowsum row (psum partition 64)
                nc.vector.tensor_copy(attnT[h][:], at[0:64, :])
                stage = pBr.tile([65, ROWS], f32, tag="stage")
                nc.vector.tensor_copy(stage[64:65, :], at[64:65, :])
                # remap rowsum to partition 0 (free layout) and to [128, 8]
                nc.sync.dma_start(rs_d[h:h + 1, :], stage[64:65, :])
                rq = pBr.tile([128, 8], f32, tag="rq")
                nc.sync.dma_start(
                    rq[:], rs_d[h, :].rearrange("(qb r) -> r qb", r=128))
                recn = pBr.tile([128, 8], f32, tag="recn")
                nc.vector.reciprocal(recn[:], rq[:])
                # normalize attnT[h] by 1/rowsum along free dim (bcast matmul)
                nc.gpsimd.dma_start(
                    rc_d[h, :].rearrange("(qb r) -> r qb", r=128), recn[:])
                rc16 = pBr.tile([1, ROWS], bf16, tag="rt16")
                nc.sync.dma_start(rc16[:], rc_d[h:h + 1, :])
                bc = psAt.tile([128, ROWS], f32, tag="attn")
                for qs in range(2):
                    nc.tensor.matmul(
                        bc[0:64, 512 * qs:512 * qs + 512], ones[:, 0:64],
                        rc16[:, 512 * qs:512 * qs + 512], start=True, stop=True)
                nc.vector.tensor_mul(attnT[h][:], attnT[h][:], bc[0:64, :])

                # ---- N side: scores natural -> exp -> normalize -> HBM ----
                ctx_n = nc.named_scope("B_Nside"); ctx_n.__enter__()
                for qb in range(8):
                    wt = pB.tile([128, S], bf16, tag="wt")
                    for kh in range(2):
                        ps = psNp.tile([128, 1024], f32, tag="scN")
                        for kc in range(2):
                            nc.tensor.matmul(
                                ps[:, 512 * kc:512 * kc + 512],
                                qhT[p][st:st + 64, 128 * qb:128 * qb + 128],
                                khT[p][st:st + 64,
                                       1024 * kh + 512 * kc:1024 * kh + 512 * kc + 512],
                                start=True, stop=True)
                        nc.scalar.activation(
                            wt[:, 1024 * kh:1024 * kh + 1024], ps[:],
                            Act.Exp, scale=0.125)
                    nc.vector.tensor_scalar_mul(wt[:], wt[:], recn[:, qb:qb + 1])
                    nc.gpsimd.dma_start(
                        aw_d[h, 128 * qb:128 * qb + 128, :], wt[:])
                ctx_n.__exit__(None, None, None)

        # ============== Phase C: O-projection + residual + LN ==============
        # (scope below)
        with tc.tile_pool(name="pC", bufs=2) as pC, \
             tc.tile_pool(name="pCw", bufs=17) as pCw, \
             tc.tile_pool(name="psC", bufs=2, space="PSUM") as psC:

            Wo_t = []
            for h in range(H):
                t = pCw.tile([64, D], bf16, tag="wo", name=f"wo{h}")
                nc.gpsimd.dma_start(t[:], W_d["Wo"][64 * h:64 * h + 64, :])
                Wo_t.append(t)
            eps_t = pC.tile([128, 1], f32, tag="eps")
            nc.vector.memset(eps_t[:], LN_EPS)

            ctx_c = nc.named_scope("C_oproj"); ctx_c.__enter__()
            for qb in range(8):
                ps = psC.tile([128, D], f32, tag="oproj")
                for half in range(2):
                    sl = slice(512 * half, 512 * half + 512)
                    for h in range(H):
                        nc.tensor.matmul(
                            ps[:, sl], attnT[h][:, 128 * qb:128 * qb + 128],
                            Wo_t[h][:, sl], start=(h == 0), stop=False)
                    nc.tensor.matmul(
                        ps[:, sl], ones[:, 0:128], bias_b["bo"][:, sl],
                        start=False, stop=True)
                qres = pC.tile([128, D], f32, tag="qres")
                nc.sync.dma_start(qres[:], q_d[128 * qb:128 * qb + 128, :])
                r = pC.tile([128, D], f32, tag="r")
                nc.vector.tensor_add(r[:], qres[:], ps[:])
                # LayerNorm over free dim
                stats = pC.tile([128, 2, 6], f32, tag="stats")
                for sg in range(2):
                    nc.vector.bn_stats(stats[:, sg, :], r[:, 512 * sg:512 * sg + 512])
                mv = pC.tile([128, 2], f32, tag="mv")
                nc.vector.bn_aggr(mv[:], stats[:])
                std = pC.tile([128, 1], f32, tag="std")
                nc.scalar.activation(std[:], mv[:, 1:2], Act.Sqrt,
                                     bias=eps_t[:], scale=1.0)
                rstd = pC.tile([128, 1], f32, tag="rstd")
                nc.vector.reciprocal(rstd[:], std[:])
                o = pC.tile([128, D], f32, tag="o")
                nc.vector.tensor_scalar(
                    out=o[:], in0=r[:], scalar1=mv[:, 0:1], scalar2=rstd[:],
                    op0=mybir.AluOpType.subtract, op1=mybir.AluOpType.mult)
                nc.vector.tensor_mul(o[:], o[:], gamma_bc[:])
                nc.vector.tensor_add(o[:], o[:], beta_bc[:])
                nc.sync.dma_start(out_d[128 * qb:128 * qb + 128, :], o[:])
            ctx_c.__exit__(None, None, None)

    nc.compile()
    return nc


def kernel(q, k, v, Wq, bq, Wk, bk, Wv, bv, Wo, bo, gamma, beta):
    _install_profile_hook()
    from concourse.bass_utils import run_bass_kernel_spmd

    if "nc" not in _cache:
        _cache["nc"] = _build()
    nc = _cache["nc"]

    q = np.asarray(q, dtype=np.float32)
    k = np.asarray(k, dtype=np.float32)
    v = np.asarray(v, dtype=np.float32)
    common = {
        "Wq": np.asarray(Wq, np.float32), "Wk": np.asarray(Wk, np.float32),
        "Wv": np.asarray(Wv, np.float32), "Wo": np.asarray(Wo, np.float32),
        "bq": np.asarray(bq, np.float32).reshape(1, D),
        "bk": np.asarray(bk, np.float32).reshape(1, D),
        "bv": np.asarray(bv, np.float32).reshape(1, D),
        "bo": np.asarray(bo, np.float32).reshape(1, D),
        "gamma": np.asarray(gamma, np.float32).reshape(1, D),
        "beta": np.asarray(beta, np.float32).reshape(1, D),
    }
    in_maps = []
    for c in range(NCORES):
        b, half = divmod(c, 2)
        m = dict(common)
        m["q"] = np.ascontiguousarray(q[b, ROWS * half:ROWS * (half + 1), :])
        m["k"] = np.ascontiguousarray(k[b])
        m["v"] = np.ascontiguousarray(v[b])
        in_maps.append(m)

    res = run_bass_kernel_spmd(nc, in_maps, core_ids=list(range(NCORES)))
    kernel._last_exec_time_ns = res.exec_time_ns
    kernel._last_scope_times = res.per_core_scope_times
    kernel._last_profile_json = res.profile_json

    output = np.empty((B, S, D), np.float32)
    attn = np.empty((B, H, S, S), np.float32)
    for c in range(NCORES):
        b, half = divmod(c, 2)
        output[b, ROWS * half:ROWS * (half + 1), :] = res.results[c]["out"]
        attn[b, :, ROWS * half:ROWS * (half + 1), :] = res.results[c]["attn_w"]
    return output, attn


# revision 16
# speedup vs baseline: 1.4140x; 1.4140x over previous
"""Trainium2 Bass kernel for nn_MultiHeadAttention (B=4, S=2048, D=1024, H=16).

Sharding: 8 cores = (batch b, qrow-half). Each core computes a full MHA block
for its 1024 query rows of batch b: QKV projections, dense softmax attention
(returning the full attention-weight matrix), output projection, residual +
LayerNorm. No collectives; host concatenates shards.
"""
import contextlib
import ctypes
import os
import sys
import types

sys.path.insert(0, "/opt/trn_rl_repo")

import numpy as np

B, S, D, H, DK = 4, 2048, 1024, 16, 64
ROWS = 1024          # q rows per core
NCORES = 8
LN_EPS = 1e-5

_cache = {}


# ---------------------------------------------------------------------------
# NTFF profile hook (so BASS_TRACE=1 works under axon); degrades silently.
# ---------------------------------------------------------------------------
def _install_profile_hook():
    try:
        import antenv
        if "antenv.axon_hooks" in sys.modules:
            return
        store = {"hook": None}

        def _make_hook(so_path):
            try:
                lib = ctypes.CDLL(so_path)
            except OSError:
                return None
            if not hasattr(lib, "axon_start_nrt_profile"):
                return None
            lib.axon_start_nrt_profile.argtypes = [
                ctypes.POINTER(ctypes.c_int64), ctypes.c_size_t]
            lib.axon_start_nrt_profile.restype = ctypes.c_int64
            lib.axon_stop_nrt_profile.argtypes = [ctypes.c_char_p]
            lib.axon_stop_nrt_profile.restype = ctypes.c_int64

            @contextlib.contextmanager
            def _hook(output_dir, device_ids):
                import jax
                jax.devices()
                if device_ids:
                    ids = (ctypes.c_int64 * len(device_ids))(*device_ids)
                    rc = lib.axon_start_nrt_profile(ids, len(device_ids))
                else:
                    rc = lib.axon_start_nrt_profile(None, 0)
                if rc != 0:
                    raise RuntimeError(f"axon_start_nrt_profile rc={rc}")
                try:
                    yield
                finally:
                    lib.axon_stop_nrt_profile(str(output_dir).encode())

            return _hook

        mod = types.ModuleType("antenv.axon_hooks")
        mod.set_axon_ntff_profile_hook = lambda h: store.__setitem__("hook", h)
        mod.get_axon_ntff_profile_hook = lambda: store["hook"]
        sys.modules["antenv.axon_hooks"] = mod
        antenv.axon_hooks = mod
        store["hook"] = _make_hook("/opt/axon/libaxon_pjrt.so")
    except Exception:
        pass


def _build():
    import concourse.bass as bass
    import concourse.mybir as mybir
    import concourse.tile as tile
    from concourse import bacc
    from contextlib import ExitStack

    f32 = mybir.dt.float32
    bf16 = mybir.dt.bfloat16
    Act = mybir.ActivationFunctionType

    nc = bacc.Bacc("TRN2", target_bir_lowering=False, debug=False)

    q_d = nc.dram_tensor("q", [ROWS, D], f32, kind="ExternalInput").ap()
    k_d = nc.dram_tensor("k", [S, D], f32, kind="ExternalInput").ap()
    v_d = nc.dram_tensor("v", [S, D], f32, kind="ExternalInput").ap()
    W_d = {w: nc.dram_tensor(w, [D, D], f32, kind="ExternalInput").ap()
           for w in ("Wq", "Wk", "Wv", "Wo")}
    b_d = {w: nc.dram_tensor(w, [1, D], f32, kind="ExternalInput").ap()
           for w in ("bq", "bk", "bv", "bo")}
    gamma_d = nc.dram_tensor("gamma", [1, D], f32, kind="ExternalInput").ap()
    beta_d = nc.dram_tensor("beta", [1, D], f32, kind="ExternalInput").ap()
    out_d = nc.dram_tensor("out", [ROWS, D], f32, kind="ExternalOutput").ap()
    aw_d = nc.dram_tensor("attn_w", [H, ROWS, S], f32, kind="ExternalOutput").ap()

    # bf16 bounce copies of the inputs (for xbar DMA transpose, 2-byte only)
    qb_d = nc.dram_tensor("qb16", [ROWS, D], bf16).ap()
    rs_d = nc.dram_tensor("rs_scratch", [H, ROWS], f32).ap()
    rc_d = nc.dram_tensor("rc_scratch", [H, ROWS], bf16).ap()
    kb_d = nc.dram_tensor("kb16", [S, D], bf16).ap()
    vb_d = nc.dram_tensor("vb16", [S, D], bf16).ap()

    with tile.TileContext(nc) as tc, ExitStack() as ctx:
        const = ctx.enter_context(tc.tile_pool(name="const", bufs=1))
        resid = ctx.enter_context(tc.tile_pool(name="resid", bufs=1))

        ones = const.tile([1, 512], bf16)
        nc.vector.memset(ones[:], 1.0)
        ones_f32 = const.tile([1, 1], f32)
        nc.vector.memset(ones_f32[:], 1.0)
        gamma_bc = const.tile([128, D], f32)
        nc.gpsimd.dma_start(gamma_bc[:], gamma_d.to_broadcast([128, D]))
        beta_bc = const.tile([128, D], f32)
        nc.gpsimd.dma_start(beta_bc[:], beta_d.to_broadcast([128, D]))
        bias_b = {}
        for w in ("bq", "bk", "bv", "bo"):
            t = const.tile([1, D], bf16, tag=w, name=f"{w}_b")
            nc.gpsimd.dma_start(t[:], b_d[w][:])   # cast f32->bf16
            bias_b[w] = t

        # cast inputs to bf16 in HBM (SWDGE cast, HBM->HBM)
        nc.gpsimd.dma_start(qb_d[:], q_d[:])
        nc.gpsimd.dma_start(kb_d[:], k_d[:])
        nc.gpsimd.dma_start(vb_d[:], v_d[:])

        # resident projected tensors
        qhT = [resid.tile([128, ROWS], bf16, tag=f"qhT{i}", name=f"qhT{i}") for i in range(8)]
        khT = [resid.tile([128, S], bf16, tag=f"khT{i}", name=f"khT{i}") for i in range(8)]
        vh = [resid.tile([128, H * 65], bf16, tag=f"vh{i}", name=f"vh{i}") for i in range(16)]
        attnT = [resid.tile([64, ROWS], bf16, tag=f"attnT{i}", name=f"attnT{i}") for i in range(16)]

        # ============== Phase A: projections ==============
        with tc.tile_pool(name="pA", bufs=10) as pA, \
             tc.tile_pool(name="pAw", bufs=9) as pAw, \
             tc.tile_pool(name="psA", bufs=3, space="PSUM") as psA:

            ctx_q = nc.named_scope("A_qproj"); ctx_q.__enter__()
            # --- Q: qhT[m] = (Wq[:, m].T @ q.T + bq) : [128 dims, ROWS] ---
            Wq_t = []
            for dc in range(8):
                t = pAw.tile([128, D], bf16, tag="wx", name=f"wq{dc}")
                nc.gpsimd.dma_start(t[:], W_d["Wq"][128 * dc:128 * dc + 128, :])
                Wq_t.append(t)
            for rs in range(ROWS // 512):
                qTs = []
                for dc in range(8):
                    t = pA.tile([128, 512], bf16, tag="xT", name=f"qT_{rs}_{dc}")
                    nc.sync.dma_start(
                        t[:], qb_d[512 * rs:512 * rs + 512, 128 * dc:128 * dc + 128],
                        transpose=True)
                    qTs.append(t)
                for m in range(8):
                    ps = psA.tile([128, 1024], f32, tag="psa")
                    for dc in range(8):
                        nc.tensor.matmul(
                            ps[:, 0:512], Wq_t[dc][:, 128 * m:128 * m + 128],
                            qTs[dc][:, 0:512], start=(dc == 0), stop=False)
                    nc.tensor.matmul(
                        ps[:, 0:512], bias_b["bq"][:, 128 * m:128 * m + 128],
                        ones[:, 0:512], start=False, stop=True)
                    nc.vector.tensor_copy(
                        qhT[m][:, 512 * rs:512 * rs + 512], ps[:, 0:512])

            ctx_q.__exit__(None, None, None)
            ctx_k = nc.named_scope("A_kproj"); ctx_k.__enter__()
            # --- K: khT[m] = (Wk[:, m].T @ k.T + bk) : [128 dims, S] ---
            Wk_t = []
            for dc in range(8):
                t = pAw.tile([128, D], bf16, tag="wx", name=f"wk{dc}")
                nc.gpsimd.dma_start(t[:], W_d["Wk"][128 * dc:128 * dc + 128, :])
                Wk_t.append(t)
            for ks in range(S // 512):
                kTs = []
                for dc in range(8):
                    t = pA.tile([128, 512], bf16, tag="xT", name=f"kT_{ks}_{dc}")
                    nc.sync.dma_start(
                        t[:], kb_d[512 * ks:512 * ks + 512, 128 * dc:128 * dc + 128],
                        transpose=True)
                    kTs.append(t)
                for m in range(8):
                    ps = psA.tile([128, 1024], f32, tag="psa")
                    for dc in range(8):
                        nc.tensor.matmul(
                            ps[:, 0:512], Wk_t[dc][:, 128 * m:128 * m + 128],
                            kTs[dc][:, 0:512], start=(dc == 0), stop=False)
                    nc.tensor.matmul(
                        ps[:, 0:512], bias_b["bk"][:, 128 * m:128 * m + 128],
                        ones[:, 0:512], start=False, stop=True)
                    nc.vector.tensor_copy(
                        khT[m][:, 512 * ks:512 * ks + 512], ps[:, 0:512])

            ctx_k.__exit__(None, None, None)
            ctx_v = nc.named_scope("A_vproj"); ctx_v.__enter__()
            # --- V: vh[kb][key, 65h+j] = (k? no: v @ Wv + bv), ones col at 65h+64
            Wv_t = []
            for dc in range(8):
                t = pAw.tile([128, D], bf16, tag="wx", name=f"wv{dc}")
                nc.gpsimd.dma_start(t[:], W_d["Wv"][128 * dc:128 * dc + 128, :])
                Wv_t.append(t)
            for ks in range(S // 512):
                vTs = []
                for dc in range(8):
                    t = pA.tile([128, 512], bf16, tag="xT", name=f"vT_{ks}_{dc}")
                    nc.sync.dma_start(
                        t[:], vb_d[512 * ks:512 * ks + 512, 128 * dc:128 * dc + 128],
                        transpose=True)
                    vTs.append(t)
                for j in range(4):           # key blocks within this slice
                    kb = 4 * ks + j
                    ps = psA.tile([128, 1024], f32, tag="psa")
                    for half in range(2):
                        sl = slice(512 * half, 512 * half + 512)
                        for dc in range(8):
                            nc.tensor.matmul(
                                ps[:, sl], vTs[dc][:, 128 * j:128 * j + 128],
                                Wv_t[dc][:, sl], start=(dc == 0), stop=False)
                        nc.tensor.matmul(
                            ps[:, sl], ones[:, 0:128], bias_b["bv"][:, sl],
                            start=False, stop=True)
                    # scatter into vh[kb]: per head 64 cols + ones col
                    for h in range(H):
                        nc.vector.tensor_copy(
                            vh[kb][:, 65 * h:65 * h + 64],
                            ps[:, 64 * h:64 * h + 64])
                    ones_col = vh[kb][:].rearrange(
                        "p (h c) -> p h c", c=65)[:, :, 64:65]
                    nc.vector.memset(ones_col, 1.0)

            ctx_v.__exit__(None, None, None)

        # ============== Phase B: attention ==============
        # Software pipeline: T-side (scoresT/exp/PV, PE-heavy) of head i runs
        # interleaved with N-side (scores-nat/exp/normalize/HBM, ACT-heavy) of
        # head i-1; bcast-normalize of attnT[i-1] lands at block end so its
        # small-DMA chain never blocks the PE FIFO.
        with tc.tile_pool(name="pB", bufs=4) as pB, \
             tc.tile_pool(name="pBr", bufs=3) as pBr, \
             tc.tile_pool(name="psT", bufs=2, space="PSUM") as psTp, \
             tc.tile_pool(name="psN", bufs=1, space="PSUM") as psNp, \
             tc.tile_pool(name="psAt", bufs=1, space="PSUM") as psAt:

            recn_t = {}
            rc16_t = {}

            def t_unit(h, kb, at):
                p, st = h // 2, 64 * (h % 2)
                ps = psTp.tile([128, ROWS], f32, tag="scT", name=f"scT_{h}_{kb}")
                for qs in range(2):
                    nc.tensor.matmul(
                        ps[:, 512 * qs:512 * qs + 512],
                        khT[p][st:st + 64, 128 * kb:128 * kb + 128],
                        qhT[p][st:st + 64, 512 * qs:512 * qs + 512],
                        start=True, stop=True)
                et = pB.tile([128, ROWS], bf16, tag="expT", name=f"eT_{h}_{kb}")
                nc.scalar.activation(et[:], ps[:], Act.Exp, scale=0.125)
                for qs in range(2):
                    nc.tensor.matmul(
                        at[0:65, 512 * qs:512 * qs + 512],
                        vh[kb][:, 65 * h:65 * h + 65],
                        et[:, 512 * qs:512 * qs + 512],
                        start=(kb == 0), stop=(kb == 15))

            def t_evac(h, at):
                nc.vector.tensor_copy(attnT[h][:], at[0:64, :])
                stage = pBr.tile([65, ROWS], f32, tag="stage", name=f"stg{h}")
                nc.vector.tensor_copy(stage[64:65, :], at[64:65, :])
                nc.sync.dma_start(rs_d[h:h + 1, :], stage[64:65, :])
                rq = pBr.tile([128, 8], f32, tag="rq", name=f"rq{h}")
                nc.sync.dma_start(
                    rq[:], rs_d[h, :].rearrange("(qb r) -> r qb", r=128))
                recn = pBr.tile([128, 8], f32, tag="recn", name=f"recn{h}")
                nc.vector.reciprocal(recn[:], rq[:])
                recn_t[h] = recn
                nc.gpsimd.dma_start(
                    rc_d[h, :].rearrange("(qb r) -> r qb", r=128), recn[:])
                rc16 = pBr.tile([1, ROWS], bf16, tag="rt16", name=f"rc16{h}")
                nc.sync.dma_start(rc16[:], rc_d[h:h + 1, :])
                rc16_t[h] = rc16

            def n_unit(h, qb):
                p, st = h // 2, 64 * (h % 2)
                wt = pB.tile([128, S], bf16, tag="wt", name=f"wt_{h}_{qb}")
                for kh in range(2):
                    ps = psNp.tile([128, 1024], f32, tag="scN",
                                   name=f"scN_{h}_{qb}_{kh}")
                    for kc in range(2):
                        nc.tensor.matmul(
                            ps[:, 512 * kc:512 * kc + 512],
                            qhT[p][st:st + 64, 128 * qb:128 * qb + 128],
                            khT[p][st:st + 64,
                                   1024 * kh + 512 * kc:1024 * kh + 512 * kc + 512],
                            start=True, stop=True)
                    nc.scalar.activation(
                        wt[:, 1024 * kh:1024 * kh + 1024], ps[:],
                        Act.Exp, scale=0.125)
                nc.vector.tensor_scalar_mul(wt[:], wt[:], recn_t[h][:, qb:qb + 1])
                nc.gpsimd.dma_start(
                    aw_d[h, 128 * qb:128 * qb + 128, :], wt[:])

            def bc_norm(h):
                rc16 = rc16_t.pop(h)
                bc = psAt.tile([128, ROWS], f32, tag="attn", name=f"bc{h}")
                for qs in range(2):
                    nc.tensor.matmul(
                        bc[0:64, 512 * qs:512 * qs + 512], ones[:, 0:64],
                        rc16[:, 512 * qs:512 * qs + 512], start=True, stop=True)
                nc.vector.tensor_mul(attnT[h][:], attnT[h][:], bc[0:64, :])

            for i in range(H + 1):
                at = None
                if i < H:
                    at = psAt.tile([65, ROWS], f32, tag="attn", name=f"at{i}")
                for j in range(16):
                    if i < H:
                        t_unit(i, j, at)
                    if j % 2 == 1 and i > 0 and j // 2 < 8:
                        n_unit(i - 1, j // 2)
                if i < H:
                    t_evac(i, at)
                if i > 1:
                    bc_norm(i - 2)
            bc_norm(H - 1)

        # ============== Phase C: O-projection + residual + LN ==============
        # (scope below)
        with tc.tile_pool(name="pC", bufs=2) as pC, \
             tc.tile_pool(name="pCw", bufs=17) as pCw, \
             tc.tile_pool(name="psC", bufs=2, space="PSUM") as psC:

            Wo_t = []
            for h in range(H):
                t = pCw.tile([64, D], bf16, tag="wo", name=f"wo{h}")
                nc.gpsimd.dma_start(t[:], W_d["Wo"][64 * h:64 * h + 64, :])
                Wo_t.append(t)
            eps_t = pC.tile([128, 1], f32, tag="eps")
            nc.vector.memset(eps_t[:], LN_EPS)

            ctx_c = nc.named_scope("C_oproj"); ctx_c.__enter__()
            for qb in range(8):
                ps = psC.tile([128, D], f32, tag="oproj")
                for half in range(2):
                    sl = slice(512 * half, 512 * half + 512)
                    for h in range(H):
                        nc.tensor.matmul(
                            ps[:, sl], attnT[h][:, 128 * qb:128 * qb + 128],
                            Wo_t[h][:, sl], start=(h == 0), stop=False)
                    nc.tensor.matmul(
                        ps[:, sl], ones[:, 0:128], bias_b["bo"][:, sl],
                        start=False, stop=True)
                qres = pC.tile([128, D], f32, tag="qres")
                nc.sync.dma_start(qres[:], q_d[128 * qb:128 * qb + 128, :])
                r = pC.tile([128, D], f32, tag="r")
                nc.vector.tensor_add(r[:], qres[:], ps[:])
                # LayerNorm over free dim
                stats = pC.tile([128, 2, 6], f32, tag="stats")
                for sg in range(2):
                    nc.vector.bn_stats(stats[:, sg, :], r[:, 512 * sg:512 * sg + 512])
                mv = pC.tile([128, 2], f32, tag="mv")
                nc.vector.bn_aggr(mv[:], stats[:])
                std = pC.tile([128, 1], f32, tag="std")
                nc.scalar.activation(std[:], mv[:, 1:2], Act.Sqrt,
                                     bias=eps_t[:], scale=1.0)
                rstd = pC.tile([128, 1], f32, tag="rstd")
                nc.vector.reciprocal(rstd[:], std[:])
                o = pC.tile([128, D], f32, tag="o")
                nc.vector.tensor_scalar(
                    out=o[:], in0=r[:], scalar1=mv[:, 0:1], scalar2=rstd[:],
                    op0=mybir.AluOpType.subtract, op1=mybir.AluOpType.mult)
                nc.vector.tensor_mul(o[:], o[:], gamma_bc[:])
                nc.vector.tensor_add(o[:], o[:], beta_bc[:])
                nc.sync.dma_start(out_d[128 * qb:128 * qb + 128, :], o[:])
            ctx_c.__exit__(None, None, None)

    nc.compile()
    return nc


def kernel(q, k, v, Wq, bq, Wk, bk, Wv, bv, Wo, bo, gamma, beta):
    _install_profile_hook()
    from concourse.bass_utils import run_bass_kernel_spmd

    if "nc" not in _cache:
        _cache["nc"] = _build()
    nc = _cache["nc"]

    q = np.asarray(q, dtype=np.float32)
    k = np.asarray(k, dtype=np.float32)
    v = np.asarray(v, dtype=np.float32)
    common = {
        "Wq": np.asarray(Wq, np.float32), "Wk": np.asarray(Wk, np.float32),
        "Wv": np.asarray(Wv, np.float32), "Wo": np.asarray(Wo, np.float32),
        "bq": np.asarray(bq, np.float32).reshape(1, D),
        "bk": np.asarray(bk, np.float32).reshape(1, D),
        "bv": np.asarray(bv, np.float32).reshape(1, D),
        "bo": np.asarray(bo, np.float32).reshape(1, D),
        "gamma": np.asarray(gamma, np.float32).reshape(1, D),
        "beta": np.asarray(beta, np.float32).reshape(1, D),
    }
    in_maps = []
    for c in range(NCORES):
        b, half = divmod(c, 2)
        m = dict(common)
        m["q"] = np.ascontiguousarray(q[b, ROWS * half:ROWS * (half + 1), :])
        m["k"] = np.ascontiguousarray(k[b])
        m["v"] = np.ascontiguousarray(v[b])
        in_maps.append(m)

    res = run_bass_kernel_spmd(nc, in_maps, core_ids=list(range(NCORES)))
    kernel._last_exec_time_ns = res.exec_time_ns
    kernel._last_scope_times = res.per_core_scope_times
    kernel._last_profile_json = res.profile_json

    output = np.empty((B, S, D), np.float32)
    attn = np.empty((B, H, S, S), np.float32)
    for c in range(NCORES):
        b, half = divmod(c, 2)
        output[b, ROWS * half:ROWS * (half + 1), :] = res.results[c]["out"]
        attn[b, :, ROWS * half:ROWS * (half + 1), :] = res.results[c]["attn_w"]
    return output, attn
